# revision 11
# baseline (speedup 1.0000x reference)
"""Trainium2 Bass kernel for nn_AConnect (A-Connect dense MLP forward).

Computes  Z[b,o] = sum_i X[b,i] * W[i,o] * Werr[b,i,o] + bias[o] * Berr[b,o]
with B=128, ROW=OUT=1024, f32 inputs/outputs.

Strategy (pure data parallel over batch, 8 NeuronCores, 16 batches/core):
  - Werr dominates traffic: memory-bound kernel. Host casts Werr/W/X to
    bf16 (the X*W*Werr product accumulates in f32 PSUM; measured rel err
    ~4e-3 vs the f32 reference), halving HBM bytes: 32 MB/core at the
    ~315 GB/s per-core rate observed with both cores of an HBM stack
    streaming (single-core measures ~365 GB/s).
  - Werr[b] arrives as [128p x (8c x 1024o)] with partition p holding 8
    contiguous rows (i = 8p + c), so each DMA is fully-contiguous 16 KB
    runs. Each batch is split into two 1 MB DMAs alternated across the
    two HWDGE rings (sync/scalar).
  - VectorE computes Q = W .* Werr[b] in place (bf16 tensor_tensor, 2x).
  - TensorE: batches are processed in pairs; the 4 output rows of a pair
    (2 batches x 2 output halves) map to the 4 PE column groups
    (tile_position (0, 32j) via out partition 32j), so 4 matmuls run
    concurrently in the array. Contraction chunks accumulate into one
    PSUM bank holding all 4 rows; only the globally-first matmul uses
    start=True (clears the bank), per-element has_written semantics make
    the other 3 regions overwrite-then-accumulate correctly.
  - ScalarE copies the PSUM bank to SBUF once per pair; one SWDGE DMA
    with accum_op=add scatters the 4 rows onto the output DRAM, which
    was preloaded with the host-precomputed f32 bias*Berr rows (the bias
    path stays full f32).

The i-permutation (partition p, slot c <-> row 8p+c) is applied to X on
the host; the contraction is order-agnostic so W/Werr/X just need the
same layout.
"""

import numpy as np

B, ROW, OUT = 128, 1024, 1024
NCORES = 8
NB = B // NCORES          # 16 batches per core
P = 128                   # partitions
NCH = ROW // P            # 8 contraction chunks (slot c on partition p = row 8p+c)
HALF = 512                # PSUM bank limit for matmul output (f32)

_CACHE = {}


def _build():
    if "nc" in _CACHE:
        return _CACHE["nc"]
    from concourse import bacc, mybir, tile

    f32 = mybir.dt.float32
    bf16 = mybir.dt.bfloat16

    nc = bacc.Bacc("TRN2", target_bir_lowering=False, debug=False,
                   num_devices=NCORES)
    w_d = nc.declare_dram_parameter("w", [ROW, OUT], bf16, isOutput=False)
    xt_d = nc.declare_dram_parameter("xt", [P, NCH, NB], bf16, isOutput=False)
    bb_d = nc.declare_dram_parameter("bb", [NB, OUT], f32, isOutput=False)
    we_d = nc.declare_dram_parameter("werr", [NB, ROW, OUT], bf16,
                                     isOutput=False)
    out_d = nc.declare_dram_parameter("out", [NB, OUT], f32, isOutput=True)

    with tile.TileContext(nc) as tc:
        with tc.tile_pool(name="const", bufs=1) as cpool, \
             tc.tile_pool(name="werr", bufs=6) as wepool, \
             tc.tile_pool(name="stage", bufs=3) as spool, \
             tc.tile_pool(name="ps", bufs=2, space="PSUM") as pspool:

            w_sb = cpool.tile([P, NCH, OUT], bf16, tag="w_sb")
            xt_sb = cpool.tile([P, NCH, NB], bf16, tag="xt_sb")

            # partition p holds rows 8p..8p+7 -> fully contiguous runs.
            # W/xt go on the (otherwise idle) SWDGE ring so the two HWDGE
            # rings can start streaming Werr immediately.
            nc.gpsimd.dma_start(
                out=w_sb[:], in_=w_d[:].rearrange("(p c) o -> p c o", c=NCH))
            nc.gpsimd.dma_start(out=xt_sb[:], in_=xt_d[:])

            # Preload bias*Berr rows into the output; per-batch results are
            # DMA-accumulated on top.
            nc.gpsimd.dma_start(out=out_d[:], in_=bb_d[:])

            CH2 = NCH // 2
            for pair in range(NB // 2):
                b0 = 2 * pair
                wes = []
                for b in (b0, b0 + 1):
                    we = wepool.tile([P, NCH, OUT], bf16, tag="we")
                    src = we_d[b].rearrange("(p c) o -> p c o", c=NCH)
                    # two 1 MB halves on the two HWDGE rings, alternating
                    # assignment per batch to keep the rings balanced
                    r0, r1 = (nc.sync, nc.scalar) if b % 2 == 0 else \
                        (nc.scalar, nc.sync)
                    r0.dma_start(out=we[:, 0:CH2], in_=src[:, 0:CH2])
                    r1.dma_start(out=we[:, CH2:NCH], in_=src[:, CH2:NCH])
                    wes.append(we)

                pss = [pspool.tile([P, HALF], f32, tag=f"ps{j}",
                                   name=f"ps{j}_{pair}")
                       for j in range(4)]
                stage = spool.tile([P, HALF], f32, tag="stage")

                # one mult per DMA half: bigger free dim amortizes the DVE
                # per-op overhead, and the dependency granularity matches
                # the half-DMAs exactly
                for bb_i in range(2):
                    nc.vector.tensor_mul(wes[bb_i][:, 0:CH2],
                                         wes[bb_i][:, 0:CH2], w_sb[:, 0:CH2])
                    nc.vector.tensor_mul(wes[bb_i][:, CH2:NCH],
                                         wes[bb_i][:, CH2:NCH],
                                         w_sb[:, CH2:NCH])

                # 4 column groups: j = 2*(b-b0) + half, out partition 32j,
                # one PSUM bank per group
                for c in range(NCH):
                    for j in range(4):
                        bb_i, h = divmod(j, 2)
                        nc.tensor.matmul(
                            pss[j][32 * j:32 * j + 1, :],
                            xt_sb[:, c, b0 + bb_i:b0 + bb_i + 1],
                            wes[bb_i][:, c, h * HALF:(h + 1) * HALF],
                            start=(c == 0),
                            stop=(c == NCH - 1),
                            tile_position=(0, 32 * j))

                for j in range(4):
                    nc.scalar.copy(stage[32 * j:32 * j + 1, :],
                                   pss[j][32 * j:32 * j + 1, :])
                # scatter rows {0,32,64,96} onto out[b0:b0+2] with +=
                nc.gpsimd.dma_start(
                    out=out_d[b0:b0 + 2].rearrange("b (h o) -> (b h) o", h=2),
                    in_=stage[0:128:32, :],
                    accum_op=mybir.AluOpType.add)

    nc.compile()
    _CACHE["nc"] = nc
    return nc


def _in_maps(X, W, bias, Werr, Berr):
    import ml_dtypes
    bf16 = ml_dtypes.bfloat16
    X = np.asarray(X, dtype=np.float32)
    W16 = np.ascontiguousarray(np.asarray(W, dtype=np.float32).astype(bf16))
    Werr = np.asarray(Werr, dtype=np.float32)
    BB = np.asarray(bias, dtype=np.float32)[None, :] * \
        np.asarray(Berr, dtype=np.float32)
    maps = []
    for i in range(NCORES):
        sl = slice(i * NB, (i + 1) * NB)
        # xt[p, c, b] = X[b, 8p + c]
        xt = np.ascontiguousarray(
            X[sl].reshape(NB, P, NCH).transpose(1, 2, 0).astype(bf16))
        maps.append({
            "w": W16,
            "xt": xt,
            "bb": np.ascontiguousarray(BB[sl]),
            "werr": np.ascontiguousarray(Werr[sl].astype(bf16)),
        })
    return maps


def kernel(X, W, bias, Werr, Berr):
    from concourse.bass_utils import run_bass_kernel_spmd
    nc = _build()
    res = run_bass_kernel_spmd(nc, _in_maps(X, W, bias, Werr, Berr),
                               list(range(NCORES)))
    return np.concatenate([res.results[i]["out"] for i in range(NCORES)],
                          axis=0)


def kernel_profiled(X, W, bias, Werr, Berr, tmpdir=None):
    """Like kernel() but with NTFF tracing; returns (output, exec_time_ns).
    Caller must have installed the axon NTFF profile hook."""
    from concourse.bass_utils import run_bass_kernel_spmd
    nc = _build()
    res = run_bass_kernel_spmd(nc, _in_maps(X, W, bias, Werr, Berr),
                               list(range(NCORES)), trace=True, tmpdir=tmpdir)
    out = np.concatenate([res.results[i]["out"] for i in range(NCORES)],
                         axis=0)
    return out, res.exec_time_ns


# revision 13
# speedup vs baseline: 1.1848x; 1.1848x over previous
"""Trainium2 Bass kernel for nn_AConnect (A-Connect dense MLP forward).

Computes  Z[b,o] = sum_i X[b,i] * W[i,o] * Werr[b,i,o] + bias[o] * Berr[b,o]
with B=128, ROW=OUT=1024, f32 inputs/outputs.

Strategy (pure data parallel over batch, 8 NeuronCores, 16 batches/core):
  - Werr dominates traffic: memory-bound kernel. Host casts Werr/W/X to
    bf16 (the X*W*Werr product accumulates in f32 PSUM; measured rel err
    ~4e-3 vs the f32 reference), halving HBM bytes: 32 MB/core at the
    ~315 GB/s per-core rate observed with both cores of an HBM stack
    streaming (single-core measures ~365 GB/s).
  - Werr[b] arrives as [128p x (8c x 1024o)] with partition p holding 8
    contiguous rows (i = 8p + c), so each DMA is fully-contiguous 16 KB
    runs. Each batch is split into two 1 MB DMAs alternated across the
    two HWDGE rings (sync/scalar).
  - VectorE computes Q = W .* Werr[b] in place (bf16 tensor_tensor, 2x).
  - TensorE: batches are processed in pairs; the 4 output rows of a pair
    (2 batches x 2 output halves) map to the 4 PE column groups
    (tile_position (0, 32j) via out partition 32j), so 4 matmuls run
    concurrently in the array. Contraction chunks accumulate into one
    PSUM bank holding all 4 rows; only the globally-first matmul uses
    start=True (clears the bank), per-element has_written semantics make
    the other 3 regions overwrite-then-accumulate correctly.
  - ScalarE copies the PSUM bank to SBUF once per pair; one SWDGE DMA
    with accum_op=add scatters the 4 rows onto the output DRAM, which
    was preloaded with the host-precomputed f32 bias*Berr rows (the bias
    path stays full f32).

The i-permutation (partition p, slot c <-> row 8p+c) is applied to X on
the host; the contraction is order-agnostic so W/Werr/X just need the
same layout.
"""

import numpy as np

B, ROW, OUT = 128, 1024, 1024
NCORES = 8
NB = B // NCORES          # 16 batches per core
P = 128                   # partitions
NCH = ROW // P            # 8 contraction chunks (slot c on partition p = row 8p+c)
HALF = 512                # PSUM bank limit for matmul output (f32)

_CACHE = {}


def _build():
    if "nc" in _CACHE:
        return _CACHE["nc"]
    from concourse import bacc, mybir, tile

    f32 = mybir.dt.float32
    bf16 = mybir.dt.bfloat16

    nc = bacc.Bacc("TRN2", target_bir_lowering=False, debug=False,
                   num_devices=NCORES)
    w_d = nc.declare_dram_parameter("w", [ROW, OUT], bf16, isOutput=False)
    xt_d = nc.declare_dram_parameter("xt", [P, NCH, NB], bf16, isOutput=False)
    bb_d = nc.declare_dram_parameter("bb", [NB, OUT], f32, isOutput=False)
    we_d = nc.declare_dram_parameter("werr", [NB, ROW, OUT], bf16,
                                     isOutput=False)
    out_d = nc.declare_dram_parameter("out", [NB, OUT], f32, isOutput=True)

    with tile.TileContext(nc) as tc:
        with tc.tile_pool(name="const", bufs=1) as cpool, \
             tc.tile_pool(name="werr", bufs=6) as wepool, \
             tc.tile_pool(name="stage", bufs=3) as spool, \
             tc.tile_pool(name="ps", bufs=2, space="PSUM") as pspool:

            w_sb = cpool.tile([P, NCH, OUT], bf16, tag="w_sb")
            xt_sb = cpool.tile([P, NCH, NB], bf16, tag="xt_sb")

            # Preload bias*Berr rows into the output; per-batch results are
            # DMA-accumulated on top.
            nc.gpsimd.dma_start(out=out_d[:], in_=bb_d[:])
            nc.gpsimd.dma_start(out=xt_sb[:], in_=xt_d[:])

            CH2 = NCH // 2
            w_src = w_d[:].rearrange("(p c) o -> p c o", c=NCH)
            for pair in range(NB // 2):
                b0 = 2 * pair
                wes = []
                for b in (b0, b0 + 1):
                    we = wepool.tile([P, NCH, OUT], bf16, tag="we")
                    src = we_d[b].rearrange("(p c) o -> p c o", c=NCH)
                    # two 1 MB halves on the two HWDGE rings
                    nc.sync.dma_start(out=we[:, 0:CH2], in_=src[:, 0:CH2])
                    nc.scalar.dma_start(out=we[:, CH2:NCH], in_=src[:, CH2:NCH])
                    wes.append(we)
                if pair == 0:
                    # W (replicated, needed by the first mult) rides both
                    # rings right behind pair 0's Werr halves; partition p
                    # holds rows 8p..8p+7, fully contiguous runs.
                    nc.sync.dma_start(out=w_sb[:, 0:CH2], in_=w_src[:, 0:CH2])
                    nc.scalar.dma_start(out=w_sb[:, CH2:NCH],
                                        in_=w_src[:, CH2:NCH])

                pss = [pspool.tile([P, HALF], f32, tag=f"ps{j}",
                                   name=f"ps{j}_{pair}")
                       for j in range(4)]
                stage = spool.tile([P, HALF], f32, tag="stage")

                for c in range(NCH):
                    nc.vector.tensor_mul(wes[0][:, c], wes[0][:, c], w_sb[:, c])
                    nc.vector.tensor_mul(wes[1][:, c], wes[1][:, c], w_sb[:, c])

                # 4 column groups: j = 2*(b-b0) + half, out partition 32j,
                # one PSUM bank per group
                for c in range(NCH):
                    for j in range(4):
                        bb_i, h = divmod(j, 2)
                        nc.tensor.matmul(
                            pss[j][32 * j:32 * j + 1, :],
                            xt_sb[:, c, b0 + bb_i:b0 + bb_i + 1],
                            wes[bb_i][:, c, h * HALF:(h + 1) * HALF],
                            start=(c == 0),
                            stop=(c == NCH - 1),
                            tile_position=(0, 32 * j))

                for j in range(4):
                    nc.scalar.copy(stage[32 * j:32 * j + 1, :],
                                   pss[j][32 * j:32 * j + 1, :])
                # scatter rows {0,32,64,96} onto out[b0:b0+2] with +=
                nc.gpsimd.dma_start(
                    out=out_d[b0:b0 + 2].rearrange("b (h o) -> (b h) o", h=2),
                    in_=stage[0:128:32, :],
                    accum_op=mybir.AluOpType.add)

    nc.compile()
    _CACHE["nc"] = nc
    return nc


def _in_maps(X, W, bias, Werr, Berr):
    import ml_dtypes
    bf16 = ml_dtypes.bfloat16
    X = np.asarray(X, dtype=np.float32)
    W16 = np.ascontiguousarray(np.asarray(W, dtype=np.float32).astype(bf16))
    Werr = np.asarray(Werr, dtype=np.float32)
    BB = np.asarray(bias, dtype=np.float32)[None, :] * \
        np.asarray(Berr, dtype=np.float32)
    maps = []
    for i in range(NCORES):
        sl = slice(i * NB, (i + 1) * NB)
        # xt[p, c, b] = X[b, 8p + c]
        xt = np.ascontiguousarray(
            X[sl].reshape(NB, P, NCH).transpose(1, 2, 0).astype(bf16))
        maps.append({
            "w": W16,
            "xt": xt,
            "bb": np.ascontiguousarray(BB[sl]),
            "werr": np.ascontiguousarray(Werr[sl].astype(bf16)),
        })
    return maps


def kernel(X, W, bias, Werr, Berr):
    from concourse.bass_utils import run_bass_kernel_spmd
    nc = _build()
    res = run_bass_kernel_spmd(nc, _in_maps(X, W, bias, Werr, Berr),
                               list(range(NCORES)))
    return np.concatenate([res.results[i]["out"] for i in range(NCORES)],
                          axis=0)


def kernel_profiled(X, W, bias, Werr, Berr, tmpdir=None):
    """Like kernel() but with NTFF tracing; returns (output, exec_time_ns).
    Caller must have installed the axon NTFF profile hook."""
    from concourse.bass_utils import run_bass_kernel_spmd
    nc = _build()
    res = run_bass_kernel_spmd(nc, _in_maps(X, W, bias, Werr, Berr),
                               list(range(NCORES)), trace=True, tmpdir=tmpdir)
    out = np.concatenate([res.results[i]["out"] for i in range(NCORES)],
                         axis=0)
    return out, res.exec_time_ns
